# revision 1
# baseline (speedup 1.0000x reference)
"""ConvGRU 3-node chain (gnn_message_passing) on 8 TRN2 NeuronCores.

Strategy: pure data parallelism — 1 batch item per core, weights replicated,
no collectives. Per-core kernel: channels-on-partitions, zero-padded 66x66
spatial layout in the SBUF free dimension; every 3x3 conv = 9 shifted matmuls
accumulating in PSUM over row-aligned interior chunks (8 rows x 64 cols =
512); bf16 matmul inputs, fp32 PSUM accumulation; bias + sigmoid/tanh fused
into the PSUM->SBUF drain on the scalar engine; GRU elementwise on vector.

Projection + integrator convs for all three nodes run concurrently as six
64x32 PE sub-tiles (tile_position packing); each gates conv uses the full
128x128 array; the cand convs of nodes 0/1 run as a concurrent 128x64 pair.
"""
import numpy as np

B, T, CIN, H, W = 8, 8, 3, 64, 64
PROJ, CDIM, HID, NUM_NODE = 32, 32, 64, 3
PROCESS_T = T + NUM_NODE - 1  # 10

PW = W + 2                    # padded width 66
IMG = PW * PW                 # 4356
BASEO = 2                     # image offset in the free dim (guard below)
FREE = 4360                   # free size incl guards at both ends
SWEEP_OFF = BASEO + PW        # row-1 col-0 position (GRU elementwise range)
SWEEP_LEN = H * PW            # 4224
NCH = 8                       # chunks per conv: 8 rows x 64 interior cols
RPC = H // NCH                # rows per chunk: 8
TAPS = [di * PW + dj for di in (-1, 0, 1) for dj in (-1, 0, 1)]

N_CORES = 8
_cache = {}


# ------------------------------------------------------------- host packing
def _bf16(x):
    import ml_dtypes
    return np.asarray(x).astype(ml_dtypes.bfloat16)


def _pack_taps(Wc, rows, row_off=0):
    """OIHW conv weight -> [rows, 9*O] bf16 (lhsT blocks, one per tap)."""
    O, I = Wc.shape[0], Wc.shape[1]
    out = np.zeros((rows, 9 * O), np.float32)
    for k in range(9):
        di, dj = k // 3, k % 3
        out[row_off:row_off + I, k * O:(k + 1) * O] = Wc[:, :, di, dj].T
    return _bf16(out)


def _pack_gates(Wg):
    """In-ch order [bu(32); h(64)] -> partition rows [h(64); bu(32)]."""
    Wr = np.concatenate([Wg[:, CDIM:, :, :], Wg[:, :CDIM, :, :]], axis=1)
    return _pack_taps(Wr, 96)


def _prep_inputs(inputs):
    inp = {k: np.asarray(v, np.float32) for k, v in inputs.items()}
    w = {}
    xp = np.zeros((B, PROCESS_T, CIN, H, W), np.float32)
    xp[:, :T] = inp["x"]
    xb = _bf16(xp)

    w["wp0"] = _pack_taps(inp["Win0"], 64)        # x at rows 0-2 (parts 64-66)
    w["wp1"] = _pack_taps(inp["We10"], 64)        # h0 at parts 0-63
    w["wp2"] = _pack_taps(inp["We21"], 64)        # h1 at parts 0-63
    w["wi0"] = _pack_taps(inp["Wint0"], 64, 0)    # p0 rows 0-31 of [p0;p1]
    w["wi1"] = _pack_taps(inp["Wint1"], 64, 32)   # p1 rows 32-63
    w["wi2"] = _pack_taps(inp["Wint2"], 64, 0)    # p2 rows 0-31 (parts 64-95)
    for n in range(3):
        w[f"wg{n}"] = _pack_gates(inp[f"Wg{n}"])
        w[f"wc{n}"] = _pack_gates(inp[f"Wc{n}"])

    bias = np.zeros((128, 12), np.float32)
    for n in range(3):
        bias[:, n] = inp[f"bg{n}"]                # r at 0-63, z at 64-127
        bias[0:64, 3 + n] = inp[f"bc{n}"]
    bias[0:32, 6] = inp["bin0"]
    bias[32:64, 6] = inp["be10"]
    bias[64:96, 6] = inp["be21"]
    bias[0:32, 7] = inp["bint0"]
    bias[96:128, 8] = inp["bint1"]
    bias[32:64, 9] = inp["bint2"]
    bias[64:128, 10] = inp["bc1"]                 # shifted cand1 drain
    return xb, w, bias


# ------------------------------------------------------------ kernel build
def build(n_repeat=1):
    import concourse.bass as bass
    import concourse.bacc as bacc
    import concourse.mybir as mybir
    from concourse import tile

    f32, bf16 = mybir.dt.float32, mybir.dt.bfloat16
    AF = mybir.ActivationFunctionType
    ALU = mybir.AluOpType

    nc = bacc.Bacc(None, target_bir_lowering=False)

    x_ext = nc.declare_dram_parameter("x", [PROCESS_T, CIN, H, W], bf16,
                                      isOutput=False)
    wshapes = {"wp0": (64, 9 * PROJ), "wp1": (64, 9 * PROJ), "wp2": (64, 9 * PROJ),
               "wi0": (64, 9 * CDIM), "wi1": (64, 9 * CDIM), "wi2": (64, 9 * CDIM)}
    for n in range(3):
        wshapes[f"wg{n}"] = (96, 9 * 2 * HID)
        wshapes[f"wc{n}"] = (96, 9 * HID)
    w_ext = {k: nc.declare_dram_parameter(k, list(s), bf16, isOutput=False)
             for k, s in wshapes.items()}
    bias_ext = nc.declare_dram_parameter("bias", [128, 12], f32, isOutput=False)
    out_ext = nc.declare_dram_parameter("out", [HID, H, W], f32, isOutput=True)

    with tile.TileContext(nc) as tc:
        with (
            tc.tile_pool(name="pers", bufs=1) as pers,
            tc.tile_pool(name="ps", bufs=1, space=bass.MemorySpace.PSUM) as ps,
        ):
            def ptile(nm, shape, dt):
                return pers.tile(shape, dt, name=nm, tag=nm, uniquify=False)

            S = [ptile(f"S{n}", [128, FREE], bf16) for n in range(3)]
            C = [ptile(f"C{n}", [128, FREE], bf16) for n in range(3)]
            Z = [ptile(f"Z{n}", [128, FREE], bf16) for n in range(3)]
            D = [ptile(f"D{n}", [128, FREE], bf16) for n in range(3)]
            P = ptile("P", [128, FREE], bf16)
            X = [ptile(f"X{i}", [128, FREE], bf16) for i in range(2)]
            OUTF = ptile("OUTF", [128, H * W], f32)
            WT = {k: ptile(f"w_{k}", [128, wshapes[k][1]], bf16) for k in wshapes}
            BIAS = ptile("BIAS", [128, 12], f32)

            for k in wshapes:
                r0 = 64 if k in ("wp0", "wi2") else 0
                nc.sync.dma_start(WT[k][r0:r0 + wshapes[k][0], :], w_ext[k][:])
            nc.sync.dma_start(BIAS[:], bias_ext[:])
            for tns in [P] + C + Z + D + X:
                nc.gpsimd.memset(tns[:], 0.0)

            def img3(tns, p0, p1):
                return tns[p0:p1, BASEO:BASEO + IMG].rearrange(
                    "p (r s) -> p r s", r=PW, s=PW)

            def mov(tns, p0, p1, c, d):
                """Moving AP for chunk c, tap shift d: [K, 8 rows, 64 cols]."""
                s = BASEO + (1 + RPC * c) * PW + 1 + d
                return tns[p0:p1, s:s + RPC * PW].rearrange(
                    "p (r s) -> p r s", r=RPC, s=PW)[:, :, 0:W]

            def dst(tns, p0, p1, c):
                """Drain destination: interior rows of chunk c."""
                return img3(tns, p0, p1)[:, 1 + RPC * c:1 + RPC * (c + 1), 1:1 + W]

            def q3(q, p0, p1):
                return q[p0:p1, 0:512].rearrange("p (r s) -> p r s", r=RPC, s=W)

            def qtile(nm, tag):
                return ps.tile([128, 512], f32, name=nm, tag=tag, uniquify=True)

            sw = slice(SWEEP_OFF, SWEEP_OFF + SWEEP_LEN)

            for rep in range(n_repeat):
                for n in range(3):
                    nc.gpsimd.memset(S[n][:], 0.0)

                for t in range(PROCESS_T):
                    act1, act2 = t >= 1, t >= 2
                    Xt = X[t % 2]
                    # x[t] was prefetched during t-1; prefetch x[t+1] now so
                    # the DMA hides behind this timestep's compute
                    if t == 0:
                        nc.sync.dma_start(
                            img3(X[0], 64, 64 + CIN)[:, 1:1 + H, 1:1 + W], x_ext[0])
                    if t + 1 < PROCESS_T:
                        nc.sync.dma_start(
                            img3(X[(t + 1) % 2], 64, 64 + CIN)[:, 1:1 + H, 1:1 + W],
                            x_ext[t + 1])

                    def integ_chunk(c):
                        qi0 = qtile(f"q_i0_{rep}_{t}_{c}", "qi0")
                        qi1 = qtile(f"q_i1_{rep}_{t}_{c}", "qi1")
                        qi2 = qtile(f"q_i2_{rep}_{t}_{c}", "qi2") if act2 else None
                        for k in range(9):
                            d = TAPS[k]
                            nc.tensor.matmul(   # integ0: [p0;p1] rows, out parts 0-31
                                qi0[0:32, 0:512], WT["wi0"][0:64, k * 32:k * 32 + 32],
                                mov(P, 0, 64, c, d),
                                start=(k == 0), stop=(k == 8), tile_position=(0, 0))
                            nc.tensor.matmul(   # integ1: out parts 96-127
                                qi1[96:128, 0:512], WT["wi1"][0:64, k * 32:k * 32 + 32],
                                mov(P, 0, 64, c, d),
                                start=(k == 0), stop=(k == 8), tile_position=(0, 96))
                            if act2:
                                nc.tensor.matmul(   # integ2: p2 parts 64-95, out 32-63
                                    qi2[32:64, 0:512],
                                    WT["wi2"][64:128, k * 32:k * 32 + 32],
                                    mov(P, 64, 128, c, d),
                                    start=(k == 0), stop=(k == 8),
                                    tile_position=(64, 32))
                        # drains on vector engine (partition shift to parts 64-95)
                        nc.vector.tensor_scalar_add(
                            dst(S[0], 64, 96, c), q3(qi0, 0, 32), BIAS[0:32, 7:8])
                        nc.vector.tensor_scalar_add(
                            dst(S[1], 64, 96, c), q3(qi1, 96, 128), BIAS[96:128, 8:9])
                        if act2:
                            nc.vector.tensor_scalar_add(
                                dst(S[2], 64, 96, c), q3(qi2, 32, 64), BIAS[32:64, 9:10])

                    # ---------- phase A+B: proj || integ, mode 64x32
                    # (node-2's proj/integ skipped before it activates)
                    for c in range(NCH):
                        q0 = qtile(f"q_p0_{rep}_{t}_{c}", "qp0")
                        q1 = qtile(f"q_p1_{rep}_{t}_{c}", "qp1")
                        q2 = qtile(f"q_p2_{rep}_{t}_{c}", "qp2") if act2 else None
                        for k in range(9):
                            d = TAPS[k]
                            nc.tensor.matmul(   # proj1: h0 -> p1, out parts 32-63
                                q1[32:64, 0:512], WT["wp1"][0:64, k * 32:k * 32 + 32],
                                mov(S[0], 0, 64, c, d),
                                start=(k == 0), stop=(k == 8), tile_position=(0, 32))
                            if act2:
                                nc.tensor.matmul(   # proj2: h1 -> p2, out parts 64-95
                                    q2[64:96, 0:512],
                                    WT["wp2"][0:64, k * 32:k * 32 + 32],
                                    mov(S[1], 0, 64, c, d),
                                    start=(k == 0), stop=(k == 8),
                                    tile_position=(0, 64))
                            nc.tensor.matmul(   # proj0: x -> p0, out parts 0-31
                                q0[0:32, 0:512], WT["wp0"][64:128, k * 32:k * 32 + 32],
                                mov(Xt, 64, 128, c, d),
                                start=(k == 0), stop=(k == 8), tile_position=(64, 0))
                        nc.scalar.activation(dst(P, 0, 32, c), q3(q0, 0, 32),
                                             AF.Identity, bias=BIAS[0:32, 6:7])
                        nc.scalar.activation(dst(P, 32, 64, c), q3(q1, 32, 64),
                                             AF.Identity, bias=BIAS[32:64, 6:7])
                        if act2:
                            nc.scalar.activation(dst(P, 64, 96, c), q3(q2, 64, 96),
                                                 AF.Identity, bias=BIAS[64:96, 6:7])
                        if c >= 2:
                            integ_chunk(c - 2)
                    integ_chunk(NCH - 2)
                    integ_chunk(NCH - 1)

                    # ---------- gates convs, full array, per node
                    # chunks processed in pairs with per-tap bank alternation:
                    # consecutive matmuls into one PSUM bank stall the PE, so
                    # alternate banks (and reuse each tap's stationary twice)
                    for n in range(3):
                        if (n == 1 and not act1) or (n == 2 and not act2):
                            continue
                        # bu copy first: depends only on integ drains, so the
                        # DVE runs it under the PE's gates matmuls
                        nc.vector.tensor_copy(C[n][64:96, sw], S[n][64:96, sw])
                        for cp in range(0, NCH, 2):
                            qga = qtile(f"q_g{n}_{rep}_{t}_{cp}", "qg0")
                            qgb = qtile(f"q_g{n}_{rep}_{t}_{cp + 1}", "qg1")
                            for k in range(9):
                                d = TAPS[k]
                                for qg, c in ((qga, cp), (qgb, cp + 1)):
                                    nc.tensor.matmul(
                                        qg[0:128, 0:512],
                                        WT[f"wg{n}"][0:96, k * 128:k * 128 + 128],
                                        mov(S[n], 0, 96, c, d),
                                        start=(k == 0), stop=(k == 8),
                                        tile_position=(0, 0))
                            for qg, c in ((qga, cp), (qgb, cp + 1)):
                                nc.scalar.activation(   # r
                                    dst(C[n], 0, 64, c), q3(qg, 0, 64),
                                    AF.Sigmoid, bias=BIAS[0:64, n:n + 1])
                                nc.scalar.activation(   # z: shift 64-127 -> 0-63
                                    dst(Z[n], 0, 64, c), q3(qg, 64, 128),
                                    AF.Sigmoid, bias=BIAS[64:128, n:n + 1])
                                nc.vector.tensor_tensor(   # rh in place
                                    dst(C[n], 0, 64, c), dst(C[n], 0, 64, c),
                                    dst(S[n], 0, 64, c), ALU.mult)

                    def cand_pair(ns):
                        """cand convs for the given nodes, interleaved per tap.
                        One node: chunk-paired bank alternation. Two nodes:
                        concurrent col tiles, banks alternate naturally."""
                        specs = []  # (node, colbase, tag)
                        if len(ns) == 2:
                            specs = [(ns[0], 0, "qp0"), (ns[1], 64, "qp1")]
                            for c in range(NCH):
                                qcs = [qtile(f"q_c{n}_{rep}_{t}_{c}", tg)
                                       for n, _, tg in specs]
                                for k in range(9):
                                    d = TAPS[k]
                                    for (n, cb, _), qc in zip(specs, qcs):
                                        nc.tensor.matmul(
                                            qc[cb:cb + 64, 0:512],
                                            WT[f"wc{n}"][0:96, k * 64:k * 64 + 64],
                                            mov(C[n], 0, 96, c, d),
                                            start=(k == 0), stop=(k == 8),
                                            tile_position=(0, cb))
                                for (n, cb, _), qc in zip(specs, qcs):
                                    bcol = 10 if cb == 64 else 3 + n
                                    nc.scalar.activation(
                                        dst(D[n], 0, 64, c), q3(qc, cb, cb + 64),
                                        AF.Tanh, bias=BIAS[cb:cb + 64, bcol:bcol + 1])
                        else:
                            n = ns[0]
                            for cp in range(0, NCH, 2):
                                qca = qtile(f"q_c{n}_{rep}_{t}_{cp}", "qp0")
                                qcb = qtile(f"q_c{n}_{rep}_{t}_{cp + 1}", "qp1")
                                for k in range(9):
                                    d = TAPS[k]
                                    for qc, c in ((qca, cp), (qcb, cp + 1)):
                                        nc.tensor.matmul(
                                            qc[0:64, 0:512],
                                            WT[f"wc{n}"][0:96, k * 64:k * 64 + 64],
                                            mov(C[n], 0, 96, c, d),
                                            start=(k == 0), stop=(k == 8),
                                            tile_position=(0, 0))
                                for qc, c in ((qca, cp), (qcb, cp + 1)):
                                    nc.scalar.activation(
                                        dst(D[n], 0, 64, c), q3(qc, 0, 64),
                                        AF.Tanh, bias=BIAS[0:64, 3 + n:4 + n])

                    def update(n):
                        nc.vector.tensor_tensor(C[n][0:64, sw], S[n][0:64, sw],
                                                D[n][0:64, sw], ALU.subtract)
                        nc.vector.tensor_tensor(C[n][0:64, sw], Z[n][0:64, sw],
                                                C[n][0:64, sw], ALU.mult)
                        nc.vector.tensor_tensor(S[n][0:64, sw], D[n][0:64, sw],
                                                C[n][0:64, sw], ALU.add)

                    cand_pair([0, 1] if act1 else [0])
                    update(0)
                    if act1:
                        update(1)
                    if act2:
                        cand_pair([2])
                        update(2)

                nc.vector.tensor_copy(
                    OUTF[0:64, :].rearrange("p (r s) -> p r s", r=H, s=W),
                    img3(S[2], 0, 64)[:, 1:1 + H, 1:1 + W])
                nc.sync.dma_start(
                    out_ext[:], OUTF[0:64, :].rearrange("p (r s) -> p r s", r=H, s=W))

    nc.compile()
    return nc


# ----------------------------------------------------------------- entry
def kernel(**inputs) -> np.ndarray:
    from concourse.bass_utils import run_bass_kernel_spmd
    xb, w, bias = _prep_inputs(inputs)
    if "nc" not in _cache:
        _cache["nc"] = build(1)
    nc = _cache["nc"]
    in_maps = []
    for b in range(N_CORES):
        m = {"x": np.ascontiguousarray(xb[b]), "bias": bias}
        m.update(w)
        in_maps.append(m)
    res = run_bass_kernel_spmd(nc, in_maps, core_ids=list(range(N_CORES))).results
    return np.stack([res[b]["out"] for b in range(N_CORES)]).astype(np.float32)



# revision 18
# speedup vs baseline: 1.4264x; 1.4264x over previous
"""ConvGRU 3-node chain (gnn_message_passing) on 8 TRN2 NeuronCores.

Strategy: pure data parallelism - 1 batch item per core, weights replicated,
no collectives. Per-core kernel: channels-on-partitions, zero-padded 66x66
spatial layout in the SBUF free dimension; every 3x3 conv = 9 shifted matmuls
accumulating in PSUM over row-aligned chunks (8 rows x 64 cols = 512 free);
bf16 matmul inputs, fp32 PSUM accumulation; bias + sigmoid/tanh fused into
the PSUM->SBUF drains on the scalar engine; GRU elementwise on vector.

Matmul cost on TRN2 is (moving rows) x pe_cycle regardless of stationary
size, so the win over a naive lowering is packing many small convs into few
dense streams (K<=128 stationary rows, N<=128 output cols):
  - node-0's bottom-up path (proj0+integ0) depends only on x: precomputed
    for all 10 steps in 6 startup streams, 4 timesteps packed per 128
    partitions.
  - per step: 3 gates streams (K=96,N=128); cand0+integ1 merged over
    CX0=[r0*h0; bu0; p1] (K=128,N=96); cand1+integ2 merged; cand2;
    proj1+proj2 merged over HH=[h0;h1] (K=128,N=64) computing next step's
    p1/p2.
  - dead tail work skipped (h0[8], h1[9], and their feeders are unused).
"""
import numpy as np

B, T, CIN, H, W = 8, 8, 3, 64, 64
PROJ, CDIM, HID, NUM_NODE = 32, 32, 64, 3
PROCESS_T = T + NUM_NODE - 1  # 10

PW = W + 2                    # padded width 66
IMG = PW * PW                 # 4356
BASEO = 2                     # image offset in the free dim (guard below)
FREE = 4360                   # free size incl guards at both ends
SWEEP_OFF = BASEO + PW        # row-1 col-0 position (GRU elementwise range)
SWEEP_LEN = H * PW            # 4224
NCH = 8                       # chunks per conv: 8 rows x 64 interior cols
RPC = H // NCH                # rows per chunk: 8
TAPS = [di * PW + dj for di in (-1, 0, 1) for dj in (-1, 0, 1)]

N_CORES = 8
_cache = {}


# ------------------------------------------------------------- host packing
def _bf16(x):
    import ml_dtypes
    return np.asarray(x).astype(ml_dtypes.bfloat16)


def _prep_inputs(inputs):
    inp = {k: np.asarray(v, np.float32) for k, v in inputs.items()}
    w = {}
    xp = np.zeros((B, PROCESS_T, CIN, H, W), np.float32)
    xp[:, :T] = inp["x"]
    xb = _bf16(xp)

    def pack(blocks, ncols, rows=128):
        out = np.zeros((rows, 9 * ncols), np.float32)
        for k in range(9):
            di, dj = k // 3, k % 3
            for r0, c0, Wt in blocks:
                O, I = Wt.shape[0], Wt.shape[1]
                out[r0:r0 + I, k * ncols + c0:k * ncols + c0 + O] = \
                    Wt[:, :, di, dj].T
        return _bf16(out)

    # gates: moving = S[n] = [h (0-63); bu (64-95)]; Wg in-ch order [bu; h]
    for n in range(3):
        Wg = inp[f"Wg{n}"]
        w[f"wg{n}"] = pack([(0, 0, Wg[:, CDIM:]), (64, 0, Wg[:, :CDIM])],
                           128, rows=96)
    # cand0+integ1: moving CX0 = [rh0; bu0; p1]; cols 0-63 d0, 64-95 bu1
    w["w4"] = pack([(0, 0, inp["Wc0"][:, CDIM:]), (64, 0, inp["Wc0"][:, :CDIM]),
                    (96, 64, inp["Wint1"])], 96)
    w["w5"] = pack([(0, 0, inp["Wc1"][:, CDIM:]), (64, 0, inp["Wc1"][:, :CDIM]),
                    (96, 64, inp["Wint2"])], 96)
    w["w6"] = pack([(0, 0, inp["Wc2"][:, CDIM:]), (64, 0, inp["Wc2"][:, :CDIM])],
                   64, rows=96)
    # proj12: moving HH = [h0 (0-63); h1 (64-127)]; cols 0-31 p1, 32-63 p2
    w["w7"] = pack([(0, 0, inp["We10"]), (64, 32, inp["We21"])], 64)
    # proj0 startup: x[t] at partitions 32*(t//4)+3*(t%4); out p0[t] at
    # psum parts 32*(t%4). Only t<8 needed: x[8]=x[9]=0 and h0[8]/h0[9]
    # are never consumed.
    w["wp0"] = pack([(32 * (t // 4) + 3 * (t % 4), 32 * (t % 4), inp["Win0"])
                     for t in range(T)], 128, rows=44)
    # integ0 startup: block-diagonal over 4 packed timesteps
    w["wi0"] = pack([(32 * u, 32 * u, inp["Wint0"]) for u in range(4)], 128)

    bias = np.zeros((128, 9), np.float32)
    for n in range(3):
        bias[:, n] = inp[f"bg{n}"]                    # r at 0-63, z at 64-127
    bias[0:64, 3] = inp["bc0"]; bias[64:96, 3] = inp["bint1"]
    bias[0:64, 4] = inp["bc1"]; bias[64:96, 4] = inp["bint2"]
    bias[0:64, 5] = inp["bc2"]
    bias[0:32, 6] = inp["be10"]; bias[32:64, 6] = inp["be21"]
    bias[:, 7] = np.tile(inp["bin0"], 4)
    bias[:, 8] = np.tile(inp["bint0"], 4)
    return xb, w, bias


# ------------------------------------------------------------ kernel build
def build(n_repeat=1):
    import concourse.bass as bass
    import concourse.bacc as bacc
    import concourse.mybir as mybir
    from concourse import tile

    f32, bf16 = mybir.dt.float32, mybir.dt.bfloat16
    AF = mybir.ActivationFunctionType
    ALU = mybir.AluOpType

    nc = bacc.Bacc(None, target_bir_lowering=False)

    x_ext = nc.declare_dram_parameter("x", [PROCESS_T, CIN, H, W], bf16,
                                      isOutput=False)
    wshapes = {"wg0": (96, 9 * 128), "wg1": (96, 9 * 128), "wg2": (96, 9 * 128),
               "w4": (128, 9 * 96), "w5": (128, 9 * 96), "w6": (96, 9 * 64),
               "w7": (128, 9 * 64), "wp0": (44, 9 * 128), "wi0": (128, 9 * 128)}
    w_ext = {k: nc.declare_dram_parameter(k, list(s), bf16, isOutput=False)
             for k, s in wshapes.items()}
    bias_ext = nc.declare_dram_parameter("bias", [128, 9], f32, isOutput=False)
    out_ext = nc.declare_dram_parameter("out", [HID, H, W], f32, isOutput=True)

    with tile.TileContext(nc) as tc:
        with (
            tc.tile_pool(name="pers", bufs=1) as pers,
            tc.tile_pool(name="ps", bufs=1, space=bass.MemorySpace.PSUM) as ps,
        ):
            def ptile(nm, shape, dt):
                return pers.tile(shape, dt, name=nm, tag=nm, uniquify=False)

            # S[n]: h at parts 0-63, bu at 64-95 (gates moving)
            # CX[n]: r*h at 0-63, bu at 64-95, next-p at 96-127 (cand moving)
            # ZD[n]: dense (no spatial padding), parts 0-63 only: z in free
            # [0,4096), d in [4096,8192) - DVE tensor_tensor requires both
            # inputs at the same base partition, so z, d, h all live at 0-63
            S = [ptile(f"S{n}", [128, FREE], bf16) for n in range(3)]
            CX = [ptile(f"CX{n}", [128, FREE], bf16) for n in range(3)]
            ZD = [ptile(f"ZD{n}", [64, 2 * H * W], bf16) for n in range(3)]
            HH = ptile("HH", [128, FREE], bf16)
            XA = ptile("XA", [128, FREE], bf16)
            PA = ptile("PA", [128, 2 * FREE], bf16)   # p0[t], 4 steps/img
            BA = ptile("BA", [128, 2 * FREE], bf16)   # bu0[t]
            OUTF = ptile("OUTF", [128, H * W // 2], f32)
            WT = {k: ptile(f"w_{k}", [128, wshapes[k][1]], bf16)
                  for k in wshapes}
            BIAS = ptile("BIAS", [128, 9], f32)

            for k in wshapes:
                nc.sync.dma_start(WT[k][0:wshapes[k][0], :], w_ext[k][:])
            nc.sync.dma_start(BIAS[:], bias_ext[:])
            for tns in S + CX + ZD + [HH, XA, PA, BA]:
                nc.gpsimd.memset(tns[:], 0.0)

            def img3(tns, p0, p1, img=0):
                o = img * FREE + BASEO
                return tns[p0:p1, o:o + IMG].rearrange(
                    "p (r s) -> p r s", r=PW, s=PW)

            for t in range(T):
                pb = 32 * (t // 4) + 3 * (t % 4)
                nc.sync.dma_start(img3(XA, pb, pb + 3)[:, 1:1 + H, 1:1 + W],
                                  x_ext[t])

            def mov(tns, p0, p1, c, d, img=0):
                s = img * FREE + BASEO + (1 + RPC * c) * PW + 1 + d
                return tns[p0:p1, s:s + RPC * PW].rearrange(
                    "p (r s) -> p r s", r=RPC, s=PW)[:, :, 0:W]

            def dst(tns, p0, p1, c, img=0):
                return img3(tns, p0, p1, img)[:, 1 + RPC * c:1 + RPC * (c + 1),
                                              1:1 + W]

            def q3(q, p0, p1):
                return q[p0:p1, 0:512].rearrange("p (r s) -> p r s", r=RPC, s=W)

            qn = [0]

            def qtile(tag):
                qn[0] += 1
                return ps.tile([128, 512], f32, name=f"q{qn[0]}", tag=tag,
                               uniquify=True)

            sw = slice(SWEEP_OFF, SWEEP_OFF + SWEEP_LEN)

            def swi(img):
                o = img * FREE + SWEEP_OFF
                return slice(o, o + SWEEP_LEN)

            def zv(n, c=None):
                """Dense z view of ZD[n] (chunk c or whole interior)."""
                a, b = (c * 512, c * 512 + 512) if c is not None else (0, 4096)
                return ZD[n][0:64, a:b].rearrange("p (r s) -> p r s",
                                                  r=(b - a) // W, s=W)

            def dv(n, c=None):
                a, b = (c * 512, c * 512 + 512) if c is not None else (0, 4096)
                return ZD[n][0:64, 4096 + a:4096 + b].rearrange(
                    "p (r s) -> p r s", r=(b - a) // W, s=W)

            def interior(tns, p0, p1):
                return img3(tns, p0, p1)[:, 1:1 + H, 1:1 + W]

            def chunk_pairs(tags, body, drain):
                """Chunk-paired PSUM bank alternation: taps interleave between
                two banks so the PE never accumulates back-to-back into one."""
                for cp in range(0, NCH, 2):
                    qa, qb = qtile(tags[0]), qtile(tags[1])
                    for k in range(9):
                        for q, c in ((qa, cp), (qb, cp + 1)):
                            body(q, c, k)
                    for q, c in ((qa, cp), (qb, cp + 1)):
                        drain(q, c)

            # ---------- streams
            def gates_stream(n):
                Wt = WT[f"wg{n}"]

                def body(q, c, k):
                    nc.tensor.matmul(q[0:128, 0:512],
                                     Wt[0:96, k * 128:k * 128 + 128],
                                     mov(S[n], 0, 96, c, TAPS[k]),
                                     start=(k == 0), stop=(k == 8))

                def drain(q, c):
                    nc.scalar.activation(dst(CX[n], 0, 64, c), q3(q, 0, 64),
                                         AF.Sigmoid, bias=BIAS[0:64, n:n + 1])
                    nc.scalar.activation(zv(n, c), q3(q, 64, 128),
                                         AF.Sigmoid, bias=BIAS[64:128, n:n + 1])
                    nc.vector.tensor_tensor(dst(CX[n], 0, 64, c),
                                            dst(CX[n], 0, 64, c),
                                            dst(S[n], 0, 64, c), ALU.mult)

                chunk_pairs(("qg0", "qg1"), body, drain)

            def cand_stream(n, rider):
                # n=0 rider: integ1 over p1 -> bu1 into S[1][64:96]
                # n=1 rider: integ2 over p2 -> bu2 into S[2][64:96]
                Wt = WT[("w4", "w5", "w6")[n]]
                K = 128 if rider else 96
                N = 96 if rider else 64
                ncols = 96 if n < 2 else 64

                def body(q, c, k):
                    nc.tensor.matmul(q[0:N, 0:512],
                                     Wt[0:K, k * ncols:k * ncols + N],
                                     mov(CX[n], 0, K, c, TAPS[k]),
                                     start=(k == 0), stop=(k == 8))

                def drain(q, c):
                    nc.scalar.activation(dv(n, c), q3(q, 0, 64),
                                         AF.Tanh, bias=BIAS[0:64, 3 + n:4 + n])
                    if rider:
                        nc.scalar.activation(dst(S[n + 1], 64, 96, c),
                                             q3(q, 64, 96), AF.Identity,
                                             bias=BIAS[64:96, 3 + n:4 + n])

                chunk_pairs(("qc0", "qc1"), body, drain)

            def rider_stream(n):
                # integ(n+1) alone: moving CX[n][96:128] (p), out parts 64-96
                Wt = WT[("w4", "w5")[n]]

                def body(q, c, k):
                    nc.tensor.matmul(q[64:96, 0:512],
                                     Wt[96:128, k * 96 + 64:k * 96 + 96],
                                     mov(CX[n], 96, 128, c, TAPS[k]),
                                     start=(k == 0), stop=(k == 8),
                                     tile_position=(96, 64))

                def drain(q, c):
                    nc.scalar.activation(dst(S[n + 1], 64, 96, c),
                                         q3(q, 64, 96), AF.Identity,
                                         bias=BIAS[64:96, 3 + n:4 + n])

                chunk_pairs(("qc0", "qc1"), body, drain)

            def proj12_stream():
                def body(q, c, k):
                    nc.tensor.matmul(q[0:64, 0:512],
                                     WT["w7"][0:128, k * 64:k * 64 + 64],
                                     mov(HH, 0, 128, c, TAPS[k]),
                                     start=(k == 0), stop=(k == 8))

                def drain(q, c):
                    nc.scalar.activation(dst(CX[0], 96, 128, c), q3(q, 0, 32),
                                         AF.Identity, bias=BIAS[0:32, 6:7])
                    nc.scalar.activation(dst(CX[1], 96, 128, c), q3(q, 32, 64),
                                         AF.Identity, bias=BIAS[32:64, 6:7])

                chunk_pairs(("qp0", "qp1"), body, drain)

            def proj0_stream(g):
                nt = 2 if g == 2 else 4
                pb, K, N = 32 * g, 3 * nt, 32 * nt

                def body(q, c, k):
                    nc.tensor.matmul(q[0:N, 0:512],
                                     WT["wp0"][pb:pb + K, k * 128:k * 128 + N],
                                     mov(XA, pb, pb + K, c, TAPS[k]),
                                     start=(k == 0), stop=(k == 8))

                def drain(q, c):
                    nc.scalar.activation(dst(PA, 0, N, c, img=g), q3(q, 0, N),
                                         AF.Identity, bias=BIAS[0:N, 7:8])

                chunk_pairs(("qp0", "qp1"), body, drain)

            def integ0_stream(g):
                nt = 2 if g == 2 else 4
                K = N = 32 * nt

                def body(q, c, k):
                    nc.tensor.matmul(q[0:N, 0:512],
                                     WT["wi0"][0:K, k * 128:k * 128 + N],
                                     mov(PA, 0, K, c, TAPS[k], img=g),
                                     start=(k == 0), stop=(k == 8))

                def drain(q, c):
                    nc.scalar.activation(dst(BA, 0, N, c, img=g), q3(q, 0, N),
                                         AF.Identity, bias=BIAS[0:N, 8:9])

                chunk_pairs(("qc0", "qc1"), body, drain)

            def copy_bu0(t):
                # partition-shifted SBUF->SBUF moves must go via the scalar
                # engine (DVE tensor ops require matching start partitions)
                g, u = t // 4, t % 4
                nc.scalar.activation(S[0][64:96, sw],
                                     BA[32 * u:32 * u + 32, swi(g)],
                                     AF.Identity)
                nc.vector.tensor_copy(CX[0][64:96, sw], S[0][64:96, sw])

            def upd(n):
                # h' = d + z*(h - d); CX[n] interior used as scratch (r*h
                # dead by now). All operands at base partition 0.
                ci = interior(CX[n], 0, 64)
                si = interior(S[n], 0, 64)
                nc.vector.tensor_tensor(ci, si, dv(n), ALU.subtract)
                nc.vector.tensor_tensor(ci, zv(n), ci, ALU.mult)
                nc.vector.tensor_tensor(si, dv(n), ci, ALU.add)

            # ---------- program
            for rep in range(n_repeat):
                for n in range(3):
                    nc.gpsimd.memset(S[n][0:64, :], 0.0)
                for g in range(2):
                    proj0_stream(g)
                for g in range(2):
                    integ0_stream(g)

                for t in range(PROCESS_T):
                    if t == 0:
                        copy_bu0(0)
                    if t <= 7:
                        gates_stream(0)                      # S1
                        cand_stream(0, rider=(t >= 1))       # S4
                    elif t == 8:
                        rider_stream(0)                      # S4r: bu1[8] only
                    if 1 <= t <= 8:
                        nc.vector.tensor_copy(CX[1][64:96, sw],
                                              S[1][64:96, sw])   # bu1
                        gates_stream(1)                      # S2
                    if t <= 7:
                        upd(0)
                        nc.vector.tensor_copy(HH[0:64, sw], S[0][0:64, sw])
                        if t == 0:
                            nc.scalar.activation(HH[64:128, sw],
                                                 S[1][0:64, sw], AF.Identity)
                    if 1 <= t <= 8:
                        cand_stream(1, rider=True)           # S5
                    elif t == 9:
                        rider_stream(1)                      # S5r: bu2[9]
                    if t >= 1:
                        nc.vector.tensor_copy(CX[2][64:96, sw],
                                              S[2][64:96, sw])   # bu2
                    if t >= 2:
                        gates_stream(2)                      # S3
                    if 1 <= t <= 8:
                        upd(1)
                        nc.scalar.activation(HH[64:128, sw], S[1][0:64, sw],
                                             AF.Identity)
                    if t >= 2:
                        cand_stream(2, rider=False)          # S6
                    if t <= 6:
                        copy_bu0(t + 1)
                    if t <= 8:
                        proj12_stream()                      # S7
                    if t >= 2:
                        upd(2)

                # output h2 (f32): rows 0-31 via DVE to parts 0-63, rows
                # 32-63 partition-shifted via scalar to parts 64-127
                hv = img3(S[2], 0, 64)
                nc.vector.tensor_copy(
                    OUTF[0:64, :].rearrange("p (r s) -> p r s", r=H // 2, s=W),
                    hv[:, 1:1 + H // 2, 1:1 + W])
                nc.scalar.activation(
                    OUTF[64:128, :].rearrange("p (r s) -> p r s", r=H // 2,
                                              s=W),
                    hv[:, 1 + H // 2:1 + H, 1:1 + W], AF.Identity)
                nc.sync.dma_start(
                    out_ext[:, 0:H // 2, :],
                    OUTF[0:64, :].rearrange("p (r s) -> p r s", r=H // 2, s=W))
                nc.sync.dma_start(
                    out_ext[:, H // 2:H, :],
                    OUTF[64:128, :].rearrange("p (r s) -> p r s", r=H // 2,
                                              s=W))

    nc.compile()
    return nc


# ----------------------------------------------------------------- entry
def kernel(**inputs) -> np.ndarray:
    from concourse.bass_utils import run_bass_kernel_spmd
    xb, w, bias = _prep_inputs(inputs)
    if "nc" not in _cache:
        _cache["nc"] = build(1)
    nc = _cache["nc"]
    in_maps = []
    for b in range(N_CORES):
        m = {"x": np.ascontiguousarray(xb[b]), "bias": bias}
        m.update(w)
        in_maps.append(m)
    res = run_bass_kernel_spmd(nc, in_maps, core_ids=list(range(N_CORES))).results
    return np.stack([res[b]["out"] for b in range(N_CORES)]).astype(np.float32)


# revision 33
# speedup vs baseline: 1.7854x; 1.2517x over previous
"""ConvGRU 3-node chain (gnn_message_passing) on 8 TRN2 NeuronCores.

Strategy: pure data parallelism - 1 batch item per core, weights replicated,
no collectives. Per-core kernel: channels-on-partitions, zero-padded 66x66
spatial layout in the SBUF free dimension; every 3x3 conv = 9 shifted matmuls
accumulating in PSUM over row-aligned chunks (8 rows x 64 cols = 512 free);
bf16 matmul inputs, fp32 PSUM accumulation; bias + sigmoid/tanh fused into
the PSUM->SBUF drains on the scalar engine; GRU elementwise on vector.

Matmul cost on TRN2 is (moving rows) x pe_cycle regardless of stationary
size, so the win over a naive lowering is packing many small convs into few
dense streams (K<=128 stationary rows, N<=128 output cols):
  - node-0's bottom-up path (proj0+integ0) depends only on x: precomputed
    for all 10 steps in 6 startup streams, 4 timesteps packed per 128
    partitions.
  - per step: 3 gates streams (K=96,N=128); cand0+integ1 merged over
    CX0=[r0*h0; bu0; p1] (K=128,N=96); cand1+integ2 merged; cand2;
    proj1+proj2 merged over HH=[h0;h1] (K=128,N=64) computing next step's
    p1/p2.
  - dead tail work skipped (h0[8], h1[9], and their feeders are unused).
"""
import numpy as np

B, T, CIN, H, W = 8, 8, 3, 64, 64
PROJ, CDIM, HID, NUM_NODE = 32, 32, 64, 3
PROCESS_T = T + NUM_NODE - 1  # 10

PW = W + 2                    # padded width 66
IMG = PW * PW                 # 4356
BASEO = 2                     # image offset in the free dim (guard below)
FREE = 4360                   # free size incl guards at both ends
SWEEP_OFF = BASEO + PW        # row-1 col-0 position (GRU elementwise range)
SWEEP_LEN = H * PW            # 4224
NCH = 8                       # chunks per conv: 8 rows x 64 interior cols
RPC = H // NCH                # rows per chunk: 8
TAPS = [di * PW + dj for di in (-1, 0, 1) for dj in (-1, 0, 1)]

N_CORES = 8
_cache = {}


# ------------------------------------------------------------- host packing
def _bf16(x):
    import ml_dtypes
    return np.asarray(x).astype(ml_dtypes.bfloat16)


def _prep_inputs(inputs):
    inp = {k: np.asarray(v, np.float32) for k, v in inputs.items()}
    w = {}
    xp = np.zeros((B, PROCESS_T, CIN, H, W), np.float32)
    xp[:, :T] = inp["x"]
    xb = _bf16(xp)

    def pack(blocks, ncols, rows=128):
        out = np.zeros((rows, 9 * ncols), np.float32)
        for k in range(9):
            di, dj = k // 3, k % 3
            for r0, c0, Wt in blocks:
                O, I = Wt.shape[0], Wt.shape[1]
                out[r0:r0 + I, k * ncols + c0:k * ncols + c0 + O] = \
                    Wt[:, :, di, dj].T
        return _bf16(out)

    # gates: moving = S[n] = [h (0-63); bu (64-95)]; Wg in-ch order [bu; h]
    for n in range(3):
        Wg = inp[f"Wg{n}"]
        w[f"wg{n}"] = pack([(0, 0, Wg[:, CDIM:]), (64, 0, Wg[:, :CDIM])],
                           128, rows=96)
    # cand0+integ1: moving CX0 = [rh0; bu0; p1]; cols 0-63 d0, 64-95 bu1
    w["w4"] = pack([(0, 0, inp["Wc0"][:, CDIM:]), (64, 0, inp["Wc0"][:, :CDIM]),
                    (96, 64, inp["Wint1"])], 96)
    w["w5"] = pack([(0, 0, inp["Wc1"][:, CDIM:]), (64, 0, inp["Wc1"][:, :CDIM]),
                    (96, 64, inp["Wint2"])], 96)
    w["w6"] = pack([(0, 0, inp["Wc2"][:, CDIM:]), (64, 0, inp["Wc2"][:, :CDIM])],
                   64, rows=96)
    # proj12: moving HH = [h1 (0-63); h0 (64-127)]; cols 0-31 p1, 32-63 p2.
    # h1 sits at parts 0-63 so its per-step refresh is an unshifted DVE copy
    # (h1 lands late in the step; h0 is early and can go via DMA).
    w["w7"] = pack([(0, 32, inp["We21"]), (64, 0, inp["We10"])], 64)
    # proj0 startup: x[t] at partitions 32*(t//4)+3*(t%4); out p0[t] at
    # psum parts 32*(t%4). Only t<8 needed: x[8]=x[9]=0 and h0[8]/h0[9]
    # are never consumed.
    w["wp0"] = pack([(32 * (t // 4) + 3 * (t % 4), 32 * (t % 4), inp["Win0"])
                     for t in range(T)], 128, rows=44)
    # integ0 startup: block-diagonal over 4 packed timesteps
    w["wi0"] = pack([(32 * u, 32 * u, inp["Wint0"]) for u in range(4)], 128)

    bias = np.zeros((128, 9), np.float32)
    for n in range(3):
        bias[:, n] = inp[f"bg{n}"]                    # r at 0-63, z at 64-127
    bias[0:64, 3] = inp["bc0"]; bias[64:96, 3] = inp["bint1"]
    bias[0:64, 4] = inp["bc1"]; bias[64:96, 4] = inp["bint2"]
    bias[0:64, 5] = inp["bc2"]
    bias[0:32, 6] = inp["be10"]; bias[32:64, 6] = inp["be21"]
    bias[:, 7] = np.tile(inp["bin0"], 4)
    bias[:, 8] = np.tile(inp["bint0"], 4)
    return xb, w, bias


# ------------------------------------------------------------ kernel build
def build(n_repeat=1):
    import concourse.bass as bass
    import concourse.bacc as bacc
    import concourse.mybir as mybir
    from concourse import tile

    f32, bf16 = mybir.dt.float32, mybir.dt.bfloat16
    AF = mybir.ActivationFunctionType
    ALU = mybir.AluOpType

    nc = bacc.Bacc(None, target_bir_lowering=False)

    x_ext = nc.declare_dram_parameter("x", [PROCESS_T, CIN, H, W], bf16,
                                      isOutput=False)
    wshapes = {"wg0": (96, 9 * 128), "wg1": (96, 9 * 128), "wg2": (96, 9 * 128),
               "w4": (128, 9 * 96), "w5": (128, 9 * 96), "w6": (96, 9 * 64),
               "w7": (128, 9 * 64), "wp0": (44, 9 * 128), "wi0": (128, 9 * 128)}
    w_ext = {k: nc.declare_dram_parameter(k, list(s), bf16, isOutput=False)
             for k, s in wshapes.items()}
    bias_ext = nc.declare_dram_parameter("bias", [128, 9], f32, isOutput=False)
    out_ext = nc.declare_dram_parameter("out", [HID, H, W], f32, isOutput=True)

    with tile.TileContext(nc) as tc:
        with (
            tc.tile_pool(name="pers", bufs=1) as pers,
            tc.tile_pool(name="ps", bufs=1, space=bass.MemorySpace.PSUM) as ps,
        ):
            def ptile(nm, shape, dt):
                return pers.tile(shape, dt, name=nm, tag=nm, uniquify=False)

            # S[n]: h at parts 0-63, bu at 64-95 (gates moving)
            # CX[n]: r*h at 0-63, bu at 64-95, next-p at 96-127 (cand moving)
            # ZD[n]: dense (no spatial padding), parts 0-63 only: z in free
            # [0,4096), d in [4096,8192) - DVE tensor_tensor requires both
            # inputs at the same base partition, so z, d, h all live at 0-63
            S = [ptile(f"S{n}", [128, FREE], bf16) for n in range(3)]
            CX = [ptile(f"CX{n}", [128, FREE], bf16) for n in range(3)]
            ZD = [ptile(f"ZD{n}", [64, 2 * H * W], bf16) for n in range(3)]
            HH = ptile("HH", [128, FREE], bf16)
            XA = ptile("XA", [128, FREE], bf16)
            PA = ptile("PA", [128, 2 * FREE], bf16)   # p0[t], 4 steps/img
            BA = ptile("BA", [128, 2 * FREE], bf16)   # bu0[t]
            OUTF = ptile("OUTF", [128, H * W // 2], f32)
            WT = {k: ptile(f"w_{k}", [128, wshapes[k][1]], bf16)
                  for k in wshapes}
            BIAS = ptile("BIAS", [128, 9], f32)

            for k in wshapes:
                nc.sync.dma_start(WT[k][0:wshapes[k][0], :], w_ext[k][:])
            nc.sync.dma_start(BIAS[:], bias_ext[:])
            for tns in S + CX + ZD + [HH, XA, PA, BA]:
                nc.gpsimd.memset(tns[:], 0.0)

            def img3(tns, p0, p1, img=0):
                o = img * FREE + BASEO
                return tns[p0:p1, o:o + IMG].rearrange(
                    "p (r s) -> p r s", r=PW, s=PW)

            for t in range(T):
                pb = 32 * (t // 4) + 3 * (t % 4)
                nc.sync.dma_start(img3(XA, pb, pb + 3)[:, 1:1 + H, 1:1 + W],
                                  x_ext[t])

            def mov(tns, p0, p1, c, d, img=0):
                s = img * FREE + BASEO + (1 + RPC * c) * PW + 1 + d
                return tns[p0:p1, s:s + RPC * PW].rearrange(
                    "p (r s) -> p r s", r=RPC, s=PW)[:, :, 0:W]

            def dst(tns, p0, p1, c, img=0):
                return img3(tns, p0, p1, img)[:, 1 + RPC * c:1 + RPC * (c + 1),
                                              1:1 + W]

            def q3(q, p0, p1):
                return q[p0:p1, 0:512].rearrange("p (r s) -> p r s", r=RPC, s=W)

            qn = [0]

            def qtile(tag):
                qn[0] += 1
                return ps.tile([128, 512], f32, name=f"q{qn[0]}", tag=tag,
                               uniquify=True)

            sw = slice(SWEEP_OFF, SWEEP_OFF + SWEEP_LEN)

            def swi(img):
                o = img * FREE + SWEEP_OFF
                return slice(o, o + SWEEP_LEN)

            def zv(n, r0=0, r1=H):
                """Dense z view of ZD[n], interior rows r0..r1."""
                return ZD[n][0:64, r0 * W:r1 * W].rearrange(
                    "p (r s) -> p r s", r=r1 - r0, s=W)

            def dv(n, r0=0, r1=H):
                return ZD[n][0:64, 4096 + r0 * W:4096 + r1 * W].rearrange(
                    "p (r s) -> p r s", r=r1 - r0, s=W)

            def interior(tns, p0, p1, r0=0, r1=H):
                return img3(tns, p0, p1)[:, 1 + r0:1 + r1, 1:1 + W]

            def rsw(r0, r1):
                """Padded-layout free slice covering interior rows r0..r1."""
                o = BASEO + (1 + r0) * PW
                return slice(o, o + (r1 - r0) * PW)

            def chunk_quads(tags, body, drain):
                """4-bank rotation: each tap's stationary feeds 4 consecutive
                matmuls (amortizes LDWEIGHTS), banks alternate so the PE never
                accumulates back-to-back into one, and group g's drains overlap
                group g+1's matmuls."""
                for g in range(2):
                    qs = [qtile(t) for t in tags]
                    for k in range(9):
                        for i in range(4):
                            body(qs[i], 4 * g + i, k)
                    for i in range(4):
                        drain(qs[i], 4 * g + i)

            QG = ("qg0", "qg1", "qg2", "qg3")
            QC = ("qc0", "qc1", "qc2", "qc3")

            # ---------- streams
            def gates_stream(n):
                Wt = WT[f"wg{n}"]

                def body(q, c, k):
                    nc.tensor.matmul(q[0:128, 0:512],
                                     Wt[0:96, k * 128:k * 128 + 128],
                                     mov(S[n], 0, 96, c, TAPS[k]),
                                     start=(k == 0), stop=(k == 8))

                def drain(q, c):
                    nc.scalar.activation(dst(CX[n], 0, 64, c), q3(q, 0, 64),
                                         AF.Sigmoid, bias=BIAS[0:64, n:n + 1])
                    nc.scalar.activation(zv(n, RPC * c, RPC * c + RPC),
                                         q3(q, 64, 128),
                                         AF.Sigmoid, bias=BIAS[64:128, n:n + 1])
                    nc.vector.tensor_tensor(dst(CX[n], 0, 64, c),
                                            dst(CX[n], 0, 64, c),
                                            dst(S[n], 0, 64, c), ALU.mult)

                chunk_quads(QG, body, drain)

            def cand_stream(n, rider):
                # n=0 rider: integ1 over p1 -> bu1 into S[1][64:96]
                # n=1 rider: integ2 over p2 -> bu2 into S[2][64:96]
                Wt = WT[("w4", "w5", "w6")[n]]
                K = 128 if rider else 96
                N = 96 if rider else 64
                ncols = 96 if n < 2 else 64

                def body(q, c, k):
                    nc.tensor.matmul(q[0:N, 0:512],
                                     Wt[0:K, k * ncols:k * ncols + N],
                                     mov(CX[n], 0, K, c, TAPS[k]),
                                     start=(k == 0), stop=(k == 8))

                def drain(q, c):
                    nc.scalar.activation(dv(n, RPC * c, RPC * c + RPC),
                                         q3(q, 0, 64),
                                         AF.Tanh, bias=BIAS[0:64, 3 + n:4 + n])
                    if rider:
                        nc.vector.tensor_scalar_add(
                            dst(S[n + 1], 64, 96, c), q3(q, 64, 96),
                            BIAS[64:96, 3 + n:4 + n])

                chunk_quads(QC, body, drain)

            def rider_stream(n):
                # integ(n+1) alone: moving CX[n][96:128] (p), out parts 64-96
                Wt = WT[("w4", "w5")[n]]

                def body(q, c, k):
                    nc.tensor.matmul(q[64:96, 0:512],
                                     Wt[96:128, k * 96 + 64:k * 96 + 96],
                                     mov(CX[n], 96, 128, c, TAPS[k]),
                                     start=(k == 0), stop=(k == 8),
                                     tile_position=(96, 64))

                def drain(q, c):
                    nc.vector.tensor_scalar_add(
                        dst(S[n + 1], 64, 96, c), q3(q, 64, 96),
                        BIAS[64:96, 3 + n:4 + n])

                chunk_quads(QC, body, drain)

            def proj12_stream():
                def body(q, c, k):
                    nc.tensor.matmul(q[0:64, 0:512],
                                     WT["w7"][0:128, k * 64:k * 64 + 64],
                                     mov(HH, 0, 128, c, TAPS[k]),
                                     start=(k == 0), stop=(k == 8))

                def drain(q, c):
                    nc.scalar.activation(dst(CX[0], 96, 128, c), q3(q, 0, 32),
                                         AF.Identity, bias=BIAS[0:32, 6:7])
                    nc.scalar.activation(dst(CX[1], 96, 128, c), q3(q, 32, 64),
                                         AF.Identity, bias=BIAS[32:64, 6:7])

                chunk_quads(QG, body, drain)

            def proj0_stream(g):
                nt = 2 if g == 2 else 4
                pb, K, N = 32 * g, 3 * nt, 32 * nt

                def body(q, c, k):
                    nc.tensor.matmul(q[0:N, 0:512],
                                     WT["wp0"][pb:pb + K, k * 128:k * 128 + N],
                                     mov(XA, pb, pb + K, c, TAPS[k]),
                                     start=(k == 0), stop=(k == 8))

                def drain(q, c):
                    nc.scalar.activation(dst(PA, 0, N, c, img=g), q3(q, 0, N),
                                         AF.Identity, bias=BIAS[0:N, 7:8])

                chunk_quads(QG, body, drain)

            def integ0_stream(g):
                nt = 2 if g == 2 else 4
                K = N = 32 * nt

                def body(q, c, k):
                    nc.tensor.matmul(q[0:N, 0:512],
                                     WT["wi0"][0:K, k * 128:k * 128 + N],
                                     mov(PA, 0, K, c, TAPS[k], img=g),
                                     start=(k == 0), stop=(k == 8))

                def drain(q, c):
                    nc.scalar.activation(dst(BA, 0, N, c, img=g), q3(q, 0, N),
                                         AF.Identity, bias=BIAS[0:N, 8:9])

                chunk_quads(QC, body, drain)

            def copy_bu0(t):
                # partition-shifted SBUF->SBUF moves go via DMA (engines are
                # otherwise idle; DVE tensor ops need matching start partitions)
                g, u = t // 4, t % 4
                nc.sync.dma_start(S[0][64:96, sw],
                                  BA[32 * u:32 * u + 32, swi(g)])
                nc.sync.dma_start(CX[0][64:96, sw],
                                  BA[32 * u:32 * u + 32, swi(g)])

            def upd(n, eng=None, r0=0, r1=H):
                # h' = d + z*(h - d); CX[n] interior used as scratch (r*h
                # dead by now). All operands at base partition 0.
                eng = eng or nc.vector
                ci = interior(CX[n], 0, 64, r0, r1)
                si = interior(S[n], 0, 64, r0, r1)
                eng.tensor_tensor(ci, si, dv(n, r0, r1), ALU.subtract)
                eng.tensor_tensor(ci, zv(n, r0, r1), ci, ALU.mult)
                eng.tensor_tensor(si, dv(n, r0, r1), ci, ALU.add)

            # ---------- program
            for rep in range(n_repeat):
                for n in range(3):
                    nc.gpsimd.memset(S[n][0:64, :], 0.0)
                for g in range(2):
                    proj0_stream(g)
                for g in range(2):
                    integ0_stream(g)

                for t in range(PROCESS_T):
                    if t == 0:
                        copy_bu0(0)
                    if t <= 7:
                        gates_stream(0)                      # S1
                        cand_stream(0, rider=(t >= 1))       # S4
                    elif t == 8:
                        rider_stream(0)                      # S4r: bu1[8] only
                    if t <= 7:
                        for hf in (0, 1):                    # upd0 + h0->HH,
                            upd(0, r0=32 * hf, r1=32 * hf + 32)   # pipelined
                            nc.sync.dma_start(
                                HH[64:128, rsw(32 * hf, 32 * hf + 32)],
                                S[0][0:64, rsw(32 * hf, 32 * hf + 32)])
                    if t == 0:
                        nc.vector.tensor_copy(HH[0:64, sw], S[1][0:64, sw])
                    if 1 <= t <= 8:
                        nc.vector.tensor_copy(CX[1][64:96, sw],
                                              S[1][64:96, sw])   # bu1
                        gates_stream(1)                      # S2
                        cand_stream(1, rider=True)           # S5
                    elif t == 9:
                        rider_stream(1)                      # S5r: bu2[9]
                    if t >= 1:
                        nc.vector.tensor_copy(CX[2][64:96, sw],
                                              S[2][64:96, sw])   # bu2
                    if t >= 2:
                        gates_stream(2)                      # S3
                    if 1 <= t <= 8:
                        for hf in (0, 1):                    # upd1 + h1->HH
                            upd(1, r0=32 * hf, r1=32 * hf + 32)
                            nc.vector.tensor_copy(
                                HH[0:64, rsw(32 * hf, 32 * hf + 32)],
                                S[1][0:64, rsw(32 * hf, 32 * hf + 32)])
                    if t >= 2:
                        cand_stream(2, rider=False)          # S6
                    if t <= 6:
                        copy_bu0(t + 1)
                    if t <= 8:
                        proj12_stream()                      # S7
                    if t >= 2:
                        upd(2, nc.gpsimd)

                # output h2 (f32): rows 0-31 via DVE to parts 0-63, rows
                # 32-63 partition-shifted via scalar to parts 64-127
                hv = img3(S[2], 0, 64)
                nc.vector.tensor_copy(
                    OUTF[0:64, :].rearrange("p (r s) -> p r s", r=H // 2, s=W),
                    hv[:, 1:1 + H // 2, 1:1 + W])
                nc.scalar.activation(
                    OUTF[64:128, :].rearrange("p (r s) -> p r s", r=H // 2,
                                              s=W),
                    hv[:, 1 + H // 2:1 + H, 1:1 + W], AF.Identity)
                nc.sync.dma_start(
                    out_ext[:, 0:H // 2, :],
                    OUTF[0:64, :].rearrange("p (r s) -> p r s", r=H // 2, s=W))
                nc.sync.dma_start(
                    out_ext[:, H // 2:H, :],
                    OUTF[64:128, :].rearrange("p (r s) -> p r s", r=H // 2,
                                              s=W))

    nc.compile()
    return nc


# ----------------------------------------------------------------- entry
def kernel(**inputs) -> np.ndarray:
    from concourse.bass_utils import run_bass_kernel_spmd
    xb, w, bias = _prep_inputs(inputs)
    if "nc" not in _cache:
        _cache["nc"] = build(1)
    nc = _cache["nc"]
    in_maps = []
    for b in range(N_CORES):
        m = {"x": np.ascontiguousarray(xb[b]), "bias": bias}
        m.update(w)
        in_maps.append(m)
    res = run_bass_kernel_spmd(nc, in_maps, core_ids=list(range(N_CORES))).results
    return np.stack([res[b]["out"] for b in range(N_CORES)]).astype(np.float32)


# revision 37
# speedup vs baseline: 2.1525x; 1.2056x over previous
"""ConvGRU 3-node chain (gnn_message_passing) on 8 TRN2 NeuronCores.

Strategy: pure data parallelism - 1 batch item per core, weights replicated,
no collectives. Per-core kernel: channels-on-partitions, zero-padded 66x66
spatial layout in the SBUF free dimension; every 3x3 conv = 9 shifted matmuls
accumulating in PSUM over row-aligned chunks (8 rows x 64 cols = 512 free);
bf16 matmul inputs, fp32 PSUM accumulation; bias + sigmoid/tanh fused into
the PSUM->SBUF drains on the scalar engine; GRU elementwise on vector.

Matmul cost on TRN2 is (moving rows) x pe_cycle regardless of stationary
size, so the win over a naive lowering is packing many small convs into few
dense streams (K<=128 stationary rows, N<=128 output cols):
  - node-0's bottom-up path (proj0+integ0) depends only on x: precomputed
    for all 10 steps in 6 startup streams, 4 timesteps packed per 128
    partitions.
  - per step: 3 gates streams (K=96,N=128); cand0+integ1 merged over
    CX0=[r0*h0; bu0; p1] (K=128,N=96); cand1+integ2 merged; cand2;
    proj1+proj2 merged over HH=[h0;h1] (K=128,N=64) computing next step's
    p1/p2.
  - dead tail work skipped (h0[8], h1[9], and their feeders are unused).
"""
import numpy as np

B, T, CIN, H, W = 8, 8, 3, 64, 64
PROJ, CDIM, HID, NUM_NODE = 32, 32, 64, 3
PROCESS_T = T + NUM_NODE - 1  # 10

PW = W + 2                    # padded width 66
IMG = PW * PW                 # 4356
BASEO = 2                     # image offset in the free dim (guard below)
FREE = 4360                   # free size incl guards at both ends
SWEEP_OFF = BASEO + PW        # row-1 col-0 position (GRU elementwise range)
SWEEP_LEN = H * PW            # 4224
NCH = 8                       # chunks per conv: 8 rows x 64 interior cols
RPC = H // NCH                # rows per chunk: 8
TAPS = [di * PW + dj for di in (-1, 0, 1) for dj in (-1, 0, 1)]

N_CORES = 8
_cache = {}


# ------------------------------------------------------------- host packing
def _bf16(x):
    import ml_dtypes
    return np.asarray(x).astype(ml_dtypes.bfloat16)


def _prep_inputs(inputs):
    inp = {k: np.asarray(v, np.float32) for k, v in inputs.items()}
    w = {}
    xp = np.zeros((B, PROCESS_T, CIN, H, W), np.float32)
    xp[:, :T] = inp["x"]
    xb = _bf16(xp)

    def pack(blocks, ncols, rows=128):
        out = np.zeros((rows, 9 * ncols), np.float32)
        for k in range(9):
            di, dj = k // 3, k % 3
            for r0, c0, Wt in blocks:
                O, I = Wt.shape[0], Wt.shape[1]
                out[r0:r0 + I, k * ncols + c0:k * ncols + c0 + O] = \
                    Wt[:, :, di, dj].T
        return _bf16(out)

    # gates: moving = S[n] = [h (0-63); bu (64-95)]; Wg in-ch order [bu; h]
    for n in range(3):
        Wg = inp[f"Wg{n}"]
        w[f"wg{n}"] = pack([(0, 0, Wg[:, CDIM:]), (64, 0, Wg[:, :CDIM])],
                           128, rows=96)
    # cand0+integ1: moving CX0 = [rh0; bu0; p1]; cols 0-63 d0, 64-95 bu1
    w["w4"] = pack([(0, 0, inp["Wc0"][:, CDIM:]), (64, 0, inp["Wc0"][:, :CDIM]),
                    (96, 64, inp["Wint1"])], 96)
    w["w5"] = pack([(0, 0, inp["Wc1"][:, CDIM:]), (64, 0, inp["Wc1"][:, :CDIM]),
                    (96, 64, inp["Wint2"])], 96)
    w["w6"] = pack([(0, 0, inp["Wc2"][:, CDIM:]), (64, 0, inp["Wc2"][:, :CDIM])],
                   64, rows=96)
    # proj12: moving HH = [h1 (0-63); h0 (64-127)]; cols 0-31 p1, 32-63 p2.
    # h1 sits at parts 0-63 so its per-step refresh is an unshifted DVE copy
    # (h1 lands late in the step; h0 is early and can go via DMA).
    w["w7"] = pack([(0, 32, inp["We21"]), (64, 0, inp["We10"])], 64)
    # proj0 startup: x[t] at partitions 32*(t//4)+3*(t%4); out p0[t] at
    # psum parts 32*(t%4). Only t<8 needed: x[8]=x[9]=0 and h0[8]/h0[9]
    # are never consumed.
    w["wp0"] = pack([(32 * (t // 4) + 3 * (t % 4), 32 * (t % 4), inp["Win0"])
                     for t in range(T)], 128, rows=44)
    # integ0 startup: block-diagonal over 4 packed timesteps
    w["wi0"] = pack([(32 * u, 32 * u, inp["Wint0"]) for u in range(4)], 128)

    bias = np.zeros((128, 9), np.float32)
    for n in range(3):
        bias[:, n] = inp[f"bg{n}"]                    # r at 0-63, z at 64-127
    bias[0:64, 3] = inp["bc0"]; bias[64:96, 3] = inp["bint1"]
    bias[0:64, 4] = inp["bc1"]; bias[64:96, 4] = inp["bint2"]
    bias[0:64, 5] = inp["bc2"]
    bias[64:96, 6] = inp["be10"]; bias[96:128, 6] = inp["be21"]
    bias[:, 7] = np.tile(inp["bin0"], 4)
    bias[:, 8] = np.tile(inp["bint0"], 4)
    return xb, w, bias


# ------------------------------------------------------------ kernel build
def build(n_repeat=1):
    import concourse.bass as bass
    import concourse.bacc as bacc
    import concourse.mybir as mybir
    from concourse import tile

    f32, bf16 = mybir.dt.float32, mybir.dt.bfloat16
    AF = mybir.ActivationFunctionType
    ALU = mybir.AluOpType

    nc = bacc.Bacc(None, target_bir_lowering=False)

    x_ext = nc.declare_dram_parameter("x", [PROCESS_T, CIN, H, W], bf16,
                                      isOutput=False)
    wshapes = {"wg0": (96, 9 * 128), "wg1": (96, 9 * 128), "wg2": (96, 9 * 128),
               "w4": (128, 9 * 96), "w5": (128, 9 * 96), "w6": (96, 9 * 64),
               "w7": (128, 9 * 64), "wp0": (44, 9 * 128), "wi0": (128, 9 * 128)}
    w_ext = {k: nc.declare_dram_parameter(k, list(s), bf16, isOutput=False)
             for k, s in wshapes.items()}
    bias_ext = nc.declare_dram_parameter("bias", [128, 9], f32, isOutput=False)
    out_ext = nc.declare_dram_parameter("out", [HID, H, W], f32, isOutput=True)

    with tile.TileContext(nc) as tc:
        with (
            tc.tile_pool(name="pers", bufs=1) as pers,
            tc.tile_pool(name="ps", bufs=1, space=bass.MemorySpace.PSUM) as ps,
        ):
            def ptile(nm, shape, dt):
                return pers.tile(shape, dt, name=nm, tag=nm, uniquify=False)

            # S[n]: h at parts 0-63, bu at 64-95 (gates moving)
            # CX[n]: r*h at 0-63, bu at 64-95, next-p at 96-127 (cand moving)
            # ZD[n]: dense (no spatial padding), parts 0-63 only: z in free
            # [0,4096), d in [4096,8192) - DVE tensor_tensor requires both
            # inputs at the same base partition, so z, d, h all live at 0-63
            S = [ptile(f"S{n}", [128, FREE], bf16) for n in range(3)]
            CX = [ptile(f"CX{n}", [128, FREE], bf16) for n in range(3)]
            ZD = [ptile(f"ZD{n}", [64, 2 * H * W], bf16) for n in range(3)]
            HH = ptile("HH", [128, FREE], bf16)
            XA = ptile("XA", [128, FREE], bf16)
            PA = ptile("PA", [128, 2 * FREE], bf16)   # p0[t], 4 steps/img
            BA = ptile("BA", [128, 2 * FREE], bf16)   # bu0[t]
            OUTF = ptile("OUTF", [128, H * W // 2], f32)
            WT = {k: ptile(f"w_{k}", [128, wshapes[k][1]], bf16)
                  for k in wshapes}
            BIAS = ptile("BIAS", [128, 9], f32)

            for k in wshapes:
                nc.sync.dma_start(WT[k][0:wshapes[k][0], :], w_ext[k][:])
            nc.sync.dma_start(BIAS[:], bias_ext[:])
            for tns in S + CX + ZD + [HH, XA, PA, BA]:
                nc.gpsimd.memset(tns[:], 0.0)

            def img3(tns, p0, p1, img=0):
                o = img * FREE + BASEO
                return tns[p0:p1, o:o + IMG].rearrange(
                    "p (r s) -> p r s", r=PW, s=PW)

            for t in range(T):
                pb = 32 * (t // 4) + 3 * (t % 4)
                nc.sync.dma_start(img3(XA, pb, pb + 3)[:, 1:1 + H, 1:1 + W],
                                  x_ext[t])

            def mov(tns, p0, p1, c, d, img=0):
                s = img * FREE + BASEO + (1 + RPC * c) * PW + 1 + d
                return tns[p0:p1, s:s + RPC * PW].rearrange(
                    "p (r s) -> p r s", r=RPC, s=PW)[:, :, 0:W]

            def dst(tns, p0, p1, c, img=0):
                return img3(tns, p0, p1, img)[:, 1 + RPC * c:1 + RPC * (c + 1),
                                              1:1 + W]

            def q3(q, p0, p1):
                return q[p0:p1, 0:512].rearrange("p (r s) -> p r s", r=RPC, s=W)

            qn = [0]

            def qtile(tag):
                qn[0] += 1
                return ps.tile([128, 512], f32, name=f"q{qn[0]}", tag=tag,
                               uniquify=True)

            sw = slice(SWEEP_OFF, SWEEP_OFF + SWEEP_LEN)

            def swi(img):
                o = img * FREE + SWEEP_OFF
                return slice(o, o + SWEEP_LEN)

            def zv(n, r0=0, r1=H):
                """Dense z view of ZD[n], interior rows r0..r1."""
                return ZD[n][0:64, r0 * W:r1 * W].rearrange(
                    "p (r s) -> p r s", r=r1 - r0, s=W)

            def dv(n, r0=0, r1=H):
                return ZD[n][0:64, 4096 + r0 * W:4096 + r1 * W].rearrange(
                    "p (r s) -> p r s", r=r1 - r0, s=W)

            def interior(tns, p0, p1, r0=0, r1=H):
                return img3(tns, p0, p1)[:, 1 + r0:1 + r1, 1:1 + W]

            def rsw(r0, r1):
                """Padded-layout free slice covering interior rows r0..r1."""
                o = BASEO + (1 + r0) * PW
                return slice(o, o + (r1 - r0) * PW)

            def chunk_quads(tags, body, drain):
                """4-bank rotation: each tap's stationary feeds 4 consecutive
                matmuls (amortizes LDWEIGHTS), banks alternate so the PE never
                accumulates back-to-back into one, and group g's drains overlap
                group g+1's matmuls."""
                for g in range(2):
                    qs = [qtile(t) for t in tags]
                    for k in range(9):
                        for i in range(4):
                            body(qs[i], 4 * g + i, k)
                    for i in range(4):
                        drain(qs[i], 4 * g + i)

            QG = ("qg0", "qg1", "qg2", "qg3")
            QC = ("qc0", "qc1", "qc2", "qc3")

            # ---------- streams
            def gates_stream(n):
                Wt = WT[f"wg{n}"]

                def body(q, c, k):
                    nc.tensor.matmul(q[0:128, 0:512],
                                     Wt[0:96, k * 128:k * 128 + 128],
                                     mov(S[n], 0, 96, c, TAPS[k]),
                                     start=(k == 0), stop=(k == 8))

                def drain(q, c):
                    nc.scalar.activation(dst(CX[n], 0, 64, c), q3(q, 0, 64),
                                         AF.Sigmoid, bias=BIAS[0:64, n:n + 1])
                    nc.scalar.activation(zv(n, RPC * c, RPC * c + RPC),
                                         q3(q, 64, 128),
                                         AF.Sigmoid, bias=BIAS[64:128, n:n + 1])
                    nc.vector.tensor_tensor(dst(CX[n], 0, 64, c),
                                            dst(CX[n], 0, 64, c),
                                            dst(S[n], 0, 64, c), ALU.mult)

                chunk_quads(QG, body, drain)

            def cand_stream(n, rider):
                # n=0 rider: integ1 over p1 -> bu1 into S[1][64:96]
                # n=1 rider: integ2 over p2 -> bu2 into S[2][64:96]
                Wt = WT[("w4", "w5", "w6")[n]]
                K = 128 if rider else 96
                N = 96 if rider else 64
                ncols = 96 if n < 2 else 64

                def body(q, c, k):
                    nc.tensor.matmul(q[0:N, 0:512],
                                     Wt[0:K, k * ncols:k * ncols + N],
                                     mov(CX[n], 0, K, c, TAPS[k]),
                                     start=(k == 0), stop=(k == 8))

                def drain(q, c):
                    nc.scalar.activation(dv(n, RPC * c, RPC * c + RPC),
                                         q3(q, 0, 64),
                                         AF.Tanh, bias=BIAS[0:64, 3 + n:4 + n])
                    if rider:
                        nc.vector.tensor_scalar_add(
                            dst(S[n + 1], 64, 96, c), q3(q, 64, 96),
                            BIAS[64:96, 3 + n:4 + n])

                chunk_quads(QC, body, drain)

            def rider_stream(n):
                # integ(n+1) alone: moving CX[n][96:128] (p), out parts 64-96
                Wt = WT[("w4", "w5")[n]]

                def body(q, c, k):
                    nc.tensor.matmul(q[64:96, 0:512],
                                     Wt[96:128, k * 96 + 64:k * 96 + 96],
                                     mov(CX[n], 96, 128, c, TAPS[k]),
                                     start=(k == 0), stop=(k == 8),
                                     tile_position=(96, 64))

                def drain(q, c):
                    nc.vector.tensor_scalar_add(
                        dst(S[n + 1], 64, 96, c), q3(q, 64, 96),
                        BIAS[64:96, 3 + n:4 + n])

                chunk_quads(QC, body, drain)

            def p12_body(q, c, k):
                # proj12 on PE cols 64-127 / PSUM parts 64-127 so it can
                # co-stream with cand2 (cols 0-63) when both are live
                nc.tensor.matmul(q[64:128, 0:512],
                                 WT["w7"][0:128, k * 64:k * 64 + 64],
                                 mov(HH, 0, 128, c, TAPS[k]),
                                 start=(k == 0), stop=(k == 8),
                                 tile_position=(0, 64))

            def p12_drain(q, c):
                nc.scalar.activation(dst(CX[0], 96, 128, c), q3(q, 64, 96),
                                     AF.Identity, bias=BIAS[64:96, 6:7])
                nc.scalar.activation(dst(CX[1], 96, 128, c), q3(q, 96, 128),
                                     AF.Identity, bias=BIAS[96:128, 6:7])

            def proj12_stream():
                chunk_quads(QG, p12_body, p12_drain)

            def cand2_proj12_fused():
                # interleave cand2 (QC banks, cols 0-63) with proj12 (QG
                # banks, cols 64-127) tap by tap - disjoint column groups
                Wt = WT["w6"]
                for g in range(2):
                    q6 = [qtile(t) for t in QC]
                    q7 = [qtile(t) for t in QG]
                    for k in range(9):
                        for i in range(4):
                            c = 4 * g + i
                            nc.tensor.matmul(q6[i][0:64, 0:512],
                                             Wt[0:96, k * 64:k * 64 + 64],
                                             mov(CX[2], 0, 96, c, TAPS[k]),
                                             start=(k == 0), stop=(k == 8))
                            p12_body(q7[i], c, k)
                    for i in range(4):
                        c = 4 * g + i
                        nc.scalar.activation(dv(2, RPC * c, RPC * c + RPC),
                                             q3(q6[i], 0, 64), AF.Tanh,
                                             bias=BIAS[0:64, 5:6])
                        p12_drain(q7[i], c)

            def proj0_stream(g):
                nt = 2 if g == 2 else 4
                pb, K, N = 32 * g, 3 * nt, 32 * nt

                def body(q, c, k):
                    nc.tensor.matmul(q[0:N, 0:512],
                                     WT["wp0"][pb:pb + K, k * 128:k * 128 + N],
                                     mov(XA, pb, pb + K, c, TAPS[k]),
                                     start=(k == 0), stop=(k == 8))

                def drain(q, c):
                    nc.scalar.activation(dst(PA, 0, N, c, img=g), q3(q, 0, N),
                                         AF.Identity, bias=BIAS[0:N, 7:8])

                chunk_quads(QG, body, drain)

            def integ0_stream(g):
                nt = 2 if g == 2 else 4
                K = N = 32 * nt

                def body(q, c, k):
                    nc.tensor.matmul(q[0:N, 0:512],
                                     WT["wi0"][0:K, k * 128:k * 128 + N],
                                     mov(PA, 0, K, c, TAPS[k], img=g),
                                     start=(k == 0), stop=(k == 8))

                def drain(q, c):
                    nc.scalar.activation(dst(BA, 0, N, c, img=g), q3(q, 0, N),
                                         AF.Identity, bias=BIAS[0:N, 8:9])

                chunk_quads(QC, body, drain)

            def copy_bu0(t):
                # partition-shifted SBUF->SBUF moves go via DMA (engines are
                # otherwise idle; DVE tensor ops need matching start partitions)
                g, u = t // 4, t % 4
                nc.sync.dma_start(S[0][64:96, sw],
                                  BA[32 * u:32 * u + 32, swi(g)])
                nc.sync.dma_start(CX[0][64:96, sw],
                                  BA[32 * u:32 * u + 32, swi(g)])

            def upd(n, eng=None, r0=0, r1=H):
                # h' = d + z*(h - d); CX[n] interior used as scratch (r*h
                # dead by now). All operands at base partition 0.
                eng = eng or nc.vector
                ci = interior(CX[n], 0, 64, r0, r1)
                si = interior(S[n], 0, 64, r0, r1)
                eng.tensor_tensor(ci, si, dv(n, r0, r1), ALU.subtract)
                eng.tensor_tensor(ci, zv(n, r0, r1), ci, ALU.mult)
                eng.tensor_tensor(si, dv(n, r0, r1), ci, ALU.add)

            # ---------- program
            for rep in range(n_repeat):
                for n in range(3):
                    nc.gpsimd.memset(S[n][0:64, :], 0.0)
                for g in range(2):
                    proj0_stream(g)
                for g in range(2):
                    integ0_stream(g)

                for t in range(PROCESS_T):
                    if t == 0:
                        copy_bu0(0)
                    if t <= 7:
                        gates_stream(0)                      # S1
                        cand_stream(0, rider=(t >= 1))       # S4
                    elif t == 8:
                        rider_stream(0)                      # S4r: bu1[8] only
                    if t <= 7:
                        for hf in (0, 1):                    # upd0 + h0->HH,
                            upd(0, r0=32 * hf, r1=32 * hf + 32)   # pipelined
                            nc.sync.dma_start(
                                HH[64:128, rsw(32 * hf, 32 * hf + 32)],
                                S[0][0:64, rsw(32 * hf, 32 * hf + 32)])
                    if t == 0:
                        nc.vector.tensor_copy(HH[0:64, sw], S[1][0:64, sw])
                    if 1 <= t <= 8:
                        nc.vector.tensor_copy(CX[1][64:96, sw],
                                              S[1][64:96, sw])   # bu1
                        gates_stream(1)                      # S2
                        cand_stream(1, rider=True)           # S5
                    elif t == 9:
                        rider_stream(1)                      # S5r: bu2[9]
                    if t >= 1:
                        nc.vector.tensor_copy(CX[2][64:96, sw],
                                              S[2][64:96, sw])   # bu2
                    if 1 <= t <= 8:
                        for hf in (0, 1):                    # upd1 + h1->HH
                            upd(1, r0=32 * hf, r1=32 * hf + 32)
                            nc.vector.tensor_copy(
                                HH[0:64, rsw(32 * hf, 32 * hf + 32)],
                                S[1][0:64, rsw(32 * hf, 32 * hf + 32)])
                    if t >= 2:
                        gates_stream(2)                      # S3
                    if t <= 6:
                        copy_bu0(t + 1)
                    if 2 <= t <= 8:
                        cand2_proj12_fused()                 # S6 || S7
                    elif t == 9:
                        cand_stream(2, rider=False)          # S6 alone
                    elif t <= 1:
                        proj12_stream()                      # S7 alone
                    if t >= 2:
                        upd(2, nc.gpsimd)

                # output h2 (f32): rows 0-31 via DVE to parts 0-63, rows
                # 32-63 partition-shifted via scalar to parts 64-127
                hv = img3(S[2], 0, 64)
                nc.vector.tensor_copy(
                    OUTF[0:64, :].rearrange("p (r s) -> p r s", r=H // 2, s=W),
                    hv[:, 1:1 + H // 2, 1:1 + W])
                nc.scalar.activation(
                    OUTF[64:128, :].rearrange("p (r s) -> p r s", r=H // 2,
                                              s=W),
                    hv[:, 1 + H // 2:1 + H, 1:1 + W], AF.Identity)
                nc.sync.dma_start(
                    out_ext[:, 0:H // 2, :],
                    OUTF[0:64, :].rearrange("p (r s) -> p r s", r=H // 2, s=W))
                nc.sync.dma_start(
                    out_ext[:, H // 2:H, :],
                    OUTF[64:128, :].rearrange("p (r s) -> p r s", r=H // 2,
                                              s=W))

    nc.compile()
    return nc


# ----------------------------------------------------------------- entry
def kernel(**inputs) -> np.ndarray:
    from concourse.bass_utils import run_bass_kernel_spmd
    xb, w, bias = _prep_inputs(inputs)
    if "nc" not in _cache:
        _cache["nc"] = build(1)
    nc = _cache["nc"]
    in_maps = []
    for b in range(N_CORES):
        m = {"x": np.ascontiguousarray(xb[b]), "bias": bias}
        m.update(w)
        in_maps.append(m)
    res = run_bass_kernel_spmd(nc, in_maps, core_ids=list(range(N_CORES))).results
    return np.stack([res[b]["out"] for b in range(N_CORES)]).astype(np.float32)
